# revision 8
# baseline (speedup 1.0000x reference)
"""BitNet transformer block kernel for 8 Trainium2 NeuronCores.

Sharding: data-parallel with K/V dedup. Core c handles batch c//4, token
chunk c%4 (512 tokens). Each core LN+projects K/V only for its own 512
tokens, then the 4 cores of a batch AllGather K+V (one fused 2MB fp16
collective through DRAM). Q projection overlaps the collective.

Attention (no DMA transpose of A): K is stored per-head as [65, 2048]
tiles with a ones row at row 64; Q per-head as [65, 512] with row 64 = 0.
Pass 1 computes q-major logits (contraction 65) solely for the per-query
max (single DVE reduce over [128,2048] PSUM, negated). The negated max is
XBAR-transposed into Q's row 64. Pass 2 recomputes logits K-major --
the matmul itself adds -m via the ones row -- and EXP reads PSUM and
writes transposed A directly in fp16. AV uses V with a ones column per
head so row 64 of the product is the softmax denominator. Head-level
software pipelining interleaves pass1(h) / pass2(h-1) / AV(h-2) /
normalize(h-3) so the PE stays busy (p-state) inside 8 PSUM banks.
"""
import sys

sys.path.insert(0, "/opt/trn_rl_repo")

import numpy as np
from contextlib import ExitStack

import concourse.bass as bass
import concourse.bacc as bacc
import concourse.tile as tile
from concourse import mybir
from concourse.bass_utils import run_bass_kernel_spmd

F32 = mybir.dt.float32
F32R = mybir.dt.float32r
F16 = mybir.dt.float16
AF = mybir.ActivationFunctionType
AX = mybir.AxisListType

DIM = 1024
HEADS = 16
DH = 64
FF = 4096
EPS = 1e-5
T = 2048        # tokens per batch (attention KV scope)
NQ = 512        # own tokens per core
KD = DIM // 128   # 8 feature tiles
N_CORES = 8
NB = T // 128     # 16 token blocks
VW = DH + 1       # V columns per head (64 feats + ones)

_cache = {}


def _quantize(w):
    w = w.astype(np.float32)
    return np.round(np.clip(w, -2.0, 2.0) * np.float32(0.75) + np.float32(0.5)) - np.float32(0.5)


def _prep_weights(i):
    """Host-side: quantize, fold scales/LN-params, transpose to [in, out]."""
    q = {k: _quantize(i[k]) for k in ("wq", "wk", "wv", "wo", "w1", "w2")}
    Wq = q["wq"] * i["sq"][:, None]
    Wk = q["wk"] * i["sk"][:, None]
    Wv = q["wv"] * i["sv"][:, None]
    Wo = q["wo"] * i["so"][:, None]
    W1 = q["w1"] * i["s1"][:, None]
    W2 = q["w2"] * i["s2"][:, None]
    g1, b1n = i["ln1_g"], i["ln1_b"]
    g2, b2n = i["ln2_g"], i["ln2_b"]
    s8 = np.float32(DH ** -0.5)
    out = {}
    out["wqT"] = np.ascontiguousarray((Wq * g1[None, :] * s8).T.astype(np.float16))
    out["bq"] = ((Wq @ b1n + i["bq"]) * s8).astype(np.float32)
    out["wkT"] = np.ascontiguousarray((Wk * g1[None, :]).T.astype(np.float16))
    out["bk"] = (Wk @ b1n + i["bk"]).astype(np.float32)
    out["wvT"] = np.ascontiguousarray((Wv * g1[None, :]).T.astype(np.float16))
    out["woT"] = np.ascontiguousarray(Wo.T.astype(np.float16))
    out["bo"] = (Wo @ (Wv @ b1n + i["bv"]) + i["bo"]).astype(np.float32)
    out["w1T"] = np.ascontiguousarray((W1 * g2[None, :]).T.astype(np.float16))
    out["b1"] = (W1 @ b2n + i["b1"]).astype(np.float32)
    out["w2T"] = np.ascontiguousarray(W2.T.astype(np.float16))
    out["b2"] = i["b2"].astype(np.float32)
    return out


def _ln_chunk(nc, sb, scratch, ps_stat, xh_pool, xt, ones_in, ones_sq, width,
              out_dt=F16):
    """LayerNorm transform of one feature-major chunk [128, KD, width].
    Returns xh = (x - mu) * rstd in out_dt. ones_in must match xt dtype."""
    ssum = ps_stat.tile([1, width], F32, name="ssum")
    ssq = ps_stat.tile([1, width], F32, name="ssq")
    for k in range(KD):
        sq = scratch.tile([128, width], F32R, name="scr", tag="sq")
        nc.scalar.activation(sq[:], xt[:, k], AF.Square)
        nc.tensor.matmul(ssum[:], lhsT=ones_in[:], rhs=xt[:, k],
                         start=(k == 0), stop=(k == KD - 1))
        nc.tensor.matmul(ssq[:], lhsT=ones_sq[:], rhs=sq[:],
                         start=(k == 0), stop=(k == KD - 1))
    mu = sb.tile([1, width], F32R, name="mu")
    nc.vector.tensor_scalar_mul(mu[:], ssum[:], 1.0 / DIM)
    var = sb.tile([1, width], F32, name="var")
    musq = sb.tile([1, width], F32, name="musq")
    nc.vector.tensor_mul(musq[:], mu[:], mu[:])
    nc.vector.tensor_scalar(var[:], ssq[:], 1.0 / DIM, None,
                            mybir.AluOpType.mult)
    nc.vector.tensor_sub(var[:], var[:], musq[:])
    nc.vector.tensor_scalar_add(var[:], var[:], float(EPS))
    sd = sb.tile([1, width], F32, name="sd")
    nc.scalar.activation(sd[:], var[:], AF.Sqrt)
    r = sb.tile([1, width], F32R, name="r")
    with nc.allow_low_precision(reason="f32r is fp32 storage"):
        nc.vector.reciprocal(r[:], sd[:])
    mu_b = sb.tile([128, width], F32R, name="mu_b", tag="mu_b")
    r_b = sb.tile([128, width], F32R, name="r_b", tag="r_b")
    nc.gpsimd.partition_broadcast(mu_b[:], mu[:])
    nc.gpsimd.partition_broadcast(r_b[:], r[:])
    xh = []
    for k in range(KD):
        xc = scratch.tile([128, width], F32, name="scr2", tag="xc")
        nc.vector.tensor_sub(xc[:], xt[:, k], mu_b[:])
        xhk = xh_pool.tile([128, width], out_dt, name=f"xh{k}", tag=f"xh{k}")
        nc.vector.tensor_mul(xhk[:], xc[:], r_b[:])
        xh.append(xhk)
    return xh


def _wslice(d, name, m, mm=128):
    """[DIM_in, n_out] weight dram -> lhsT tile view [128, KD_in, mm] for
    out-block m."""
    return d[name].rearrange("(k p) (mb mm) -> p k mb mm", p=128, mm=mm)[:, :, m]


KCOLS = KD * NQ            # 4096 fp16 cols of K in the staging tile
VCOLS = 4 * HEADS * VW     # 4160 fp16 cols of V (4 token blocks x 16 h x 65)


def _phase_a(nc, tc, d, Ktil, V5, Qtil, xt0, bias, ones16, ones32):
    """LN1 + K/V proj of own chunk, fused K+V AllGather, Q proj overlapped."""
    with ExitStack() as actx:
        sb_ln = actx.enter_context(tc.tile_pool(name="sb_ln", bufs=2))
        scratch = actx.enter_context(tc.tile_pool(name="scratch", bufs=2))
        sb_xh = actx.enter_context(tc.tile_pool(name="sb_xh", bufs=1))
        wstr = actx.enter_context(tc.tile_pool(name="wstr", bufs=2))
        sb_kv = actx.enter_context(tc.tile_pool(name="sb_kv", bufs=1))
        dram = actx.enter_context(tc.tile_pool(name="dram", bufs=1, space="DRAM"))
        ps_stat = actx.enter_context(tc.tile_pool(name="ps_stat", bufs=2, space="PSUM"))
        ps_mm = actx.enter_context(tc.tile_pool(name="ps_mm", bufs=4, space="PSUM"))

        stage = sb_kv.tile([128, KCOLS + VCOLS], F16, name="kv_stage")
        stage_k = stage[:, 0:KCOLS].rearrange("p (k t) -> p k t", t=NQ)
        stage_v = stage[:, KCOLS:].rearrange("p (b h e) -> p b h e", b=4, h=HEADS)
        nc.vector.memset(stage_v[:, :, :, DH:DH + 1], 1.0)

        kv_in = dram.tile([128, KCOLS + VCOLS], F16, name="kv_in")
        kv_out = dram.tile([4, 128, KCOLS + VCOLS], F16, name="kv_out")

        xh = _ln_chunk(nc, sb_ln, scratch, ps_stat, sb_xh, xt0, ones16, ones32,
                       NQ)

        # K projection of own chunk (feature-major out)
        for m in range(KD):
            wk = wstr.tile([128, KD, 128], F16, name="wk", tag="wk")
            nc.sync.dma_start(out=wk[:], in_=_wslice(d, "wkT", m))
            kp = ps_mm.tile([128, NQ], F32, name="kp", tag="mm")
            for k in range(KD):
                nc.tensor.matmul(kp[:], lhsT=wk[:, k], rhs=xh[k][:],
                                 start=(k == 0), stop=(k == KD - 1))
            nc.scalar.activation(stage_k[:, m], kp[:], AF.Identity,
                                 bias=bias["bk"][:, m:m + 1])
        # V projection of own chunk (token-major out)
        wvT_v = d["wvT"].rearrange("(kh k p) (nb nn) -> p kh k nb nn",
                                   p=128, k=4, nn=NQ)
        for nb in range(2):
            wvs = []
            for kh in range(2):
                wv = wstr.tile([128, 4, NQ], F16, name="wv", tag="wv")
                nc.sync.dma_start(out=wv[:], in_=wvT_v[:, kh, :, nb])
                wvs.append(wv)
            for t_sub in range(4):
                vp = ps_mm.tile([128, NQ], F32, name="vp", tag="mm")
                for k in range(KD):
                    nc.tensor.matmul(
                        vp[:], lhsT=xh[k][:, t_sub * 128:(t_sub + 1) * 128],
                        rhs=wvs[k // 4][:, k % 4], start=(k == 0), stop=(k == KD - 1))
                vp3 = vp.rearrange("p (hh e) -> p hh e", e=DH)
                nc.scalar.copy(
                    stage_v[:, t_sub, nb * 8:(nb + 1) * 8, 0:DH], vp3[:])

        # ship own K/V, gather the batch's full K/V
        nc.sync.dma_start(out=kv_in[:], in_=stage[:])
        nc.gpsimd.collective_compute(
            "AllGather", mybir.AluOpType.bypass,
            replica_groups=[[0, 1, 2, 3], [4, 5, 6, 7]],
            ins=[kv_in.opt()], outs=[kv_out.opt()])

        # Q projection (overlaps collective flight)
        for m in range(KD):
            wq = wstr.tile([128, KD, 128], F16, name="wq", tag="wk")
            nc.sync.dma_start(out=wq[:], in_=_wslice(d, "wqT", m))
            qp = ps_mm.tile([128, NQ], F32, name="qp", tag="mm")
            for k in range(KD):
                nc.tensor.matmul(qp[:], lhsT=wq[:, k], rhs=xh[k][:],
                                 start=(k == 0), stop=(k == KD - 1))
            nc.scalar.activation(Qtil[2 * m][0:64, :], qp[0:64, :], AF.Identity,
                                 bias=bias["bq"][0:64, m:m + 1])
            nc.scalar.activation(Qtil[2 * m + 1][0:64, :], qp[64:128, :],
                                 AF.Identity, bias=bias["bq"][64:128, m:m + 1])

        # unpack gathered K into per-head [65, T] tiles
        for h in range(HEADS):
            r0 = (h % 2) * 64
            c0 = (h // 2) * NQ
            for c in range(4):
                nc.sync.dma_start(
                    out=Ktil[h][0:64, c * NQ:(c + 1) * NQ],
                    in_=kv_out[c, r0:r0 + 64, c0:c0 + NQ])
        # unpack gathered V into [128, NB, HEADS*VW]
        nc.sync.dma_start(
            out=V5.rearrange("p (c b) e -> p c b e", c=4),
            in_=kv_out[:, :, KCOLS:].rearrange("c p (b e) -> p c b e", b=4))


def _phase_b(nc, tc, Ktil, V5, Qtil, ATTN_mbs):
    """Attention: two-pass max/exp, transposed A straight from EXP, ones-col
    denominator, 4-deep head pipeline."""
    with ExitStack() as bctx:
        psA = bctx.enter_context(tc.tile_pool(name="psA", bufs=1, space="PSUM"))
        psB = bctx.enter_context(tc.tile_pool(name="psB", bufs=2, space="PSUM"))
        psav = bctx.enter_context(tc.tile_pool(name="psav", bufs=2, space="PSUM"))
        sb_AT = bctx.enter_context(tc.tile_pool(name="sb_AT", bufs=2))
        sb_st = bctx.enter_context(tc.tile_pool(name="sb_st", bufs=2))
        sb_n = bctx.enter_context(tc.tile_pool(name="sb_n", bufs=2))

        AT = {}
        mxh = {}
        avt = {}

        def norm(h):
            av = avt.pop(h)
            denT = sb_n.tile([1, NQ], F32, name="denT", tag="denT")
            nc.scalar.copy(denT[:], av[64:65, :])
            rd = sb_n.tile([1, NQ], F32, name="rd", tag="rd")
            nc.vector.reciprocal(rd[:], denT[:])
            rdb = sb_n.tile([64, NQ], F32, name="rdb", tag="rdb")
            nc.gpsimd.partition_broadcast(rdb[:], rd[:])
            mb, r0 = h // 2, (h % 2) * 64
            nc.vector.tensor_mul(ATTN_mbs[mb][r0:r0 + 64, :], av[0:64, :],
                                 rdb[:])

        for h in range(HEADS + 3):
            if h < HEADS:
                mxh[h] = sb_st.tile([128, 128], F16, name="mx", tag="mx")
            for j in range(NB):
                if h < HEADS and j % 4 == 0:
                    qt = j // 4
                    S = psA.tile([128, T], F32, name="S", tag="S")
                    for c in range(4):
                        nc.tensor.matmul(
                            S[:, c * NQ:(c + 1) * NQ],
                            lhsT=Qtil[h][0:65, qt * 128:(qt + 1) * 128],
                            rhs=Ktil[h][0:65, c * NQ:(c + 1) * NQ],
                            start=True, stop=True)
                    nc.vector.reduce_max(mxh[h][:, qt:qt + 1], S[:],
                                         axis=AX.X, negate=True)
                if 1 <= h < HEADS + 1:
                    hp = h - 1
                    if j == 0:
                        AT[hp] = sb_AT.tile([128, NB, NQ], F16, name="AT")
                    s2 = psB.tile([128, NQ], F32, name="s2", tag="s2")
                    nc.tensor.matmul(
                        s2[:], lhsT=Ktil[hp][0:65, j * 128:(j + 1) * 128],
                        rhs=Qtil[hp][0:65, :], start=True, stop=True)
                    nc.scalar.activation(AT[hp][:, j, :], s2[:], AF.Exp)
                if 2 <= h < HEADS + 2:
                    ha = h - 2
                    if j == 0:
                        avt[ha] = psav.tile([128, NQ], F32, name="av", tag="av")
                    nc.tensor.matmul(
                        avt[ha][0:VW, :],
                        lhsT=V5[:, j, ha * VW:(ha + 1) * VW],
                        rhs=AT[ha][:, j, :],
                        start=(j == 0), stop=(j == NB - 1))
                    if j == NB - 1:
                        AT.pop(ha)
            if h < HEADS:
                # negated maxes -> row 64 of Qtil[h] (XBAR transpose)
                mT = sb_st.tile([128, 128], F16, name="mT", tag="mT")
                nc.sync.dma_start(out=mT[:], in_=mxh.pop(h)[:], transpose=True)
                for qt in range(4):
                    nc.sync.dma_start(
                        out=Qtil[h][64:65, qt * 128:(qt + 1) * 128],
                        in_=mT[qt:qt + 1, 0:128])
            if h >= 3:
                norm(h - 3)


def _phase_c(nc, tc, d, ATTN_mbs, xt0, bias, ones32):
    """O proj + residual + LN2 + FF + output store."""
    with ExitStack() as cctx:
        sb_ln2 = cctx.enter_context(tc.tile_pool(name="sb_ln2", bufs=2))
        scr2 = cctx.enter_context(tc.tile_pool(name="scr2", bufs=2))
        sb_u = cctx.enter_context(tc.tile_pool(name="sb_u", bufs=1))
        wstr2 = cctx.enter_context(tc.tile_pool(name="wstr2", bufs=4))
        ps_stat2 = cctx.enter_context(tc.tile_pool(name="ps_stat2", bufs=1, space="PSUM"))
        ps_mm2 = cctx.enter_context(tc.tile_pool(name="ps_mm2", bufs=6, space="PSUM"))

        u_sb = sb_u.tile([128, KD, NQ], F32R, name="u_sb")
        for m in range(KD):
            wot = wstr2.tile([128, KD, 128], F16, name="wo", tag="wsm")
            nc.sync.dma_start(out=wot[:], in_=_wslice(d, "woT", m))
            op = ps_mm2.tile([128, NQ], F32, name="op", tag="mm")
            for k in range(KD):
                nc.tensor.matmul(op[:], lhsT=wot[:, k], rhs=ATTN_mbs[k][:],
                                 start=(k == 0), stop=(k == KD - 1))
            upre = scr2.tile([128, NQ], F32, name="upre", tag="scr")
            nc.vector.tensor_add(upre[:], op[:], xt0[:, m])
            nc.scalar.activation(u_sb[:, m], upre[:], AF.Identity,
                                 bias=bias["bo"][:, m:m + 1])
        uh = _ln_chunk(nc, sb_ln2, scr2, ps_stat2, sb_u, u_sb, ones32,
                       ones32, NQ, out_dt=F16)
        H_sb = sb_u.tile([128, FF // 128, NQ], F16, name="H_sb")
        for m in range(FF // 128):
            w1t = wstr2.tile([128, KD, 128], F16, name="w1", tag="wsm")
            nc.sync.dma_start(out=w1t[:], in_=_wslice(d, "w1T", m))
            h1 = ps_mm2.tile([128, NQ], F32, name="h1", tag="mm")
            for k in range(KD):
                nc.tensor.matmul(h1[:], lhsT=w1t[:, k], rhs=uh[k][:],
                                 start=(k == 0), stop=(k == KD - 1))
            nc.scalar.activation(H_sb[:, m], h1[:], AF.Gelu,
                                 bias=bias["b1"][:, m:m + 1])
        w2T_v = d["w2T"].rearrange("(kh k p) (mb mm) -> p kh k mb mm",
                                   p=128, k=8, mm=128)
        for m in range(KD):
            f2 = ps_mm2.tile([128, NQ], F32, name="f2", tag="mm")
            for kh in range(4):
                w2 = wstr2.tile([128, 8, 128], F16, name="w2", tag="w2")
                nc.sync.dma_start(out=w2[:], in_=w2T_v[:, kh, :, m])
                for k in range(8):
                    nc.tensor.matmul(f2[:], lhsT=w2[:, k], rhs=H_sb[:, kh * 8 + k],
                                     start=(kh == 0 and k == 0),
                                     stop=(kh == 3 and k == 7))
            opre = scr2.tile([128, NQ], F32, name="opre", tag="scr")
            nc.vector.tensor_add(opre[:], f2[:], u_sb[:, m])
            oout = scr2.tile([128, NQ], F32, name="oout", tag="scr")
            nc.scalar.activation(oout[:], opre[:], AF.Identity,
                                 bias=bias["b2"][:, m:m + 1])
            nc.sync.dma_start(out=d["yT"][m * 128:(m + 1) * 128, :], in_=oout[:])


def _body(nc, tc, d):
    ctx = ExitStack()
    with ctx:
        const = ctx.enter_context(tc.tile_pool(name="const", bufs=1))
        ones_blk = const.tile([128, 128], F32, name="ones_blk")
        nc.vector.memset(ones_blk[:], 1.0)
        ones32 = const.tile([128, 1], F32R, name="ones32")
        nc.vector.tensor_copy(ones32[:], ones_blk[:, 0:1])
        ones16 = const.tile([128, 1], F16, name="ones16")
        nc.vector.tensor_copy(ones16[:], ones_blk[:, 0:1])

        xt0 = const.tile([128, KD, NQ], F16, name="xt0")
        nc.sync.dma_start(
            out=xt0[:], in_=d["xT"].rearrange("(k p) t -> p k t", p=128))

        bias = {}
        for nm, n in [("bq", DIM), ("bk", DIM), ("bo", DIM), ("b1", FF), ("b2", DIM)]:
            t = const.tile([128, n // 128], F32, name=f"sb_{nm}")
            nc.sync.dma_start(out=t[:], in_=d[nm].rearrange("(m p) -> p m", p=128))
            bias[nm] = t

        # long-lived activations
        ATTN_mbs = [const.tile([128, NQ], F16, name=f"ATTN_{i}") for i in range(KD)]

        with tc.tile_pool(name="attn_mem", bufs=1) as am:
            Ktil = [am.tile([128, T], F16, name=f"Kt_{h}") for h in range(HEADS)]
            Qtil = [am.tile([128, NQ], F16, name=f"Qt_{h}") for h in range(HEADS)]
            V5 = am.tile([128, NB, HEADS * VW], F16, name="V5")
            for h in range(HEADS):
                nc.vector.memset(Ktil[h][64:65, :], 1.0)
                nc.vector.memset(Qtil[h][64:65, :], 0.0)

            _phase_a(nc, tc, d, Ktil, V5, Qtil, xt0, bias, ones16, ones32)
            _phase_b(nc, tc, Ktil, V5, Qtil, ATTN_mbs)
        _phase_c(nc, tc, d, ATTN_mbs, xt0, bias, ones32)


def _build():
    nc = bacc.Bacc("TRN2", target_bir_lowering=False, debug=False,
                   num_devices=N_CORES)
    d = {}
    d["xT"] = nc.dram_tensor("xT", [DIM, NQ], F16, kind="ExternalInput").ap()
    d["wqT"] = nc.dram_tensor("wqT", [DIM, DIM], F16, kind="ExternalInput").ap()
    d["wkT"] = nc.dram_tensor("wkT", [DIM, DIM], F16, kind="ExternalInput").ap()
    d["wvT"] = nc.dram_tensor("wvT", [DIM, DIM], F16, kind="ExternalInput").ap()
    d["woT"] = nc.dram_tensor("woT", [DIM, DIM], F16, kind="ExternalInput").ap()
    d["w1T"] = nc.dram_tensor("w1T", [DIM, FF], F16, kind="ExternalInput").ap()
    d["w2T"] = nc.dram_tensor("w2T", [FF, DIM], F16, kind="ExternalInput").ap()
    for nm, n in [("bq", DIM), ("bk", DIM), ("bo", DIM), ("b1", FF), ("b2", DIM)]:
        d[nm] = nc.dram_tensor(nm, [n], F32, kind="ExternalInput").ap()
    d["yT"] = nc.dram_tensor("yT", [DIM, NQ], F32, kind="ExternalOutput").ap()
    with tile.TileContext(nc) as tc:
        _body(nc, tc, d)
    nc.compile()
    return nc


def _in_maps(inputs):
    x = inputs["x"].astype(np.float32)
    B = x.shape[0]
    w = _prep_weights(inputs)
    per_batch = N_CORES // B
    maps = []
    for c in range(N_CORES):
        b, chunk = divmod(c, per_batch)
        xT = np.ascontiguousarray(
            x[b].T[:, chunk * NQ:(chunk + 1) * NQ]).astype(np.float16)
        m = {"xT": xT}
        m.update(w)
        maps.append(m)
    return maps


def kernel(**inputs) -> np.ndarray:
    inputs = {k: np.asarray(v) for k, v in inputs.items()}
    x = inputs["x"].astype(np.float32)
    B, N, D = x.shape  # (2, 2048, 1024)

    if "nc" not in _cache:
        _cache["nc"] = _build()
    nc = _cache["nc"]

    res = run_bass_kernel_spmd(nc, _in_maps(inputs), core_ids=list(range(N_CORES)))
    per_batch = N_CORES // B
    out = np.empty((B, N, D), dtype=np.float32)
    for c in range(N_CORES):
        b, chunk = divmod(c, per_batch)
        out[b, chunk * NQ:(chunk + 1) * NQ, :] = res.results[c]["yT"].T
    return out


# revision 14
# speedup vs baseline: 1.0792x; 1.0792x over previous
"""BitNet transformer block kernel for 8 Trainium2 NeuronCores.

Sharding: data-parallel with K/V dedup. Core c handles batch c//4, token
chunk c%4 (512 tokens). Each core LN+projects K/V only for its own 512
tokens, then the 4 cores of a batch AllGather K+V (one fused 2MB fp16
collective through DRAM). Q projection overlaps the collective.

Attention (no DMA transpose of A): K is stored per-head as [65, 2048]
tiles with a ones row at row 64; Q per-head as [65, 512] with row 64 = 0.
Pass 1 computes q-major logits (contraction 65) solely for the per-query
max (single DVE reduce over [128,2048] PSUM, negated). The negated max is
XBAR-transposed into Q's row 64. Pass 2 recomputes logits K-major --
the matmul itself adds -m via the ones row -- and EXP reads PSUM and
writes transposed A directly in fp16. AV uses V with a ones column per
head so row 64 of the product is the softmax denominator. Head-level
software pipelining interleaves pass1(h) / pass2(h-1) / AV(h-2) /
normalize(h-3) so the PE stays busy (p-state) inside 8 PSUM banks.
"""
import sys

sys.path.insert(0, "/opt/trn_rl_repo")

import numpy as np
from contextlib import ExitStack

import concourse.bass as bass
import concourse.bacc as bacc
import concourse.tile as tile
from concourse import mybir
from concourse.bass_utils import run_bass_kernel_spmd

F32 = mybir.dt.float32
F32R = mybir.dt.float32r
F16 = mybir.dt.float16
AF = mybir.ActivationFunctionType
AX = mybir.AxisListType

DIM = 1024
HEADS = 16
DH = 64
FF = 4096
EPS = 1e-5
T = 2048        # tokens per batch (attention KV scope)
NQ = 512        # own tokens per core
KD = DIM // 128   # 8 feature tiles
N_CORES = 8
NB = T // 128     # 16 token blocks
VW = DH + 1       # V columns per head (64 feats + ones)

_cache = {}


def _quantize(w):
    w = w.astype(np.float32)
    return np.round(np.clip(w, -2.0, 2.0) * np.float32(0.75) + np.float32(0.5)) - np.float32(0.5)


def _prep_weights(i):
    """Host-side: quantize, fold scales/LN-params, transpose to [in, out]."""
    q = {k: _quantize(i[k]) for k in ("wq", "wk", "wv", "wo", "w1", "w2")}
    Wq = q["wq"] * i["sq"][:, None]
    Wk = q["wk"] * i["sk"][:, None]
    Wv = q["wv"] * i["sv"][:, None]
    Wo = q["wo"] * i["so"][:, None]
    W1 = q["w1"] * i["s1"][:, None]
    W2 = q["w2"] * i["s2"][:, None]
    g1, b1n = i["ln1_g"], i["ln1_b"]
    g2, b2n = i["ln2_g"], i["ln2_b"]
    s8 = np.float32(DH ** -0.5)
    out = {}
    out["wqT"] = np.ascontiguousarray((Wq * g1[None, :] * s8).T.astype(np.float16))
    out["bq"] = ((Wq @ b1n + i["bq"]) * s8).astype(np.float32)
    out["wkT"] = np.ascontiguousarray((Wk * g1[None, :]).T.astype(np.float16))
    out["bk"] = (Wk @ b1n + i["bk"]).astype(np.float32)
    out["wvT"] = np.ascontiguousarray((Wv * g1[None, :]).T.astype(np.float16))
    out["woT"] = np.ascontiguousarray(Wo.T.astype(np.float16))
    out["bo"] = (Wo @ (Wv @ b1n + i["bv"]) + i["bo"]).astype(np.float32)
    out["w1T"] = np.ascontiguousarray((W1 * g2[None, :]).T.astype(np.float16))
    out["b1"] = (W1 @ b2n + i["b1"]).astype(np.float32)
    out["w2T"] = np.ascontiguousarray(W2.T.astype(np.float16))
    out["b2"] = i["b2"].astype(np.float32)
    return out


def _ln_chunk(nc, sb, scratch, ps_stat, xh_pool, xt, ones_in, ones_sq, width,
              out_dt=F16):
    """LayerNorm transform of one feature-major chunk [128, KD, width].
    Returns xh = (x - mu) * rstd in out_dt. ones_in must match xt dtype."""
    ssum = ps_stat.tile([1, width], F32, name="ssum")
    ssq = ps_stat.tile([1, width], F32, name="ssq")
    for k in range(KD):
        sq = scratch.tile([128, width], F32R, name="scr", tag="sq")
        nc.scalar.activation(sq[:], xt[:, k], AF.Square)
        nc.tensor.matmul(ssum[:], lhsT=ones_in[:], rhs=xt[:, k],
                         start=(k == 0), stop=(k == KD - 1))
        nc.tensor.matmul(ssq[:], lhsT=ones_sq[:], rhs=sq[:],
                         start=(k == 0), stop=(k == KD - 1))
    mu = sb.tile([1, width], F32R, name="mu")
    nc.vector.tensor_scalar_mul(mu[:], ssum[:], 1.0 / DIM)
    var = sb.tile([1, width], F32, name="var")
    musq = sb.tile([1, width], F32, name="musq")
    nc.vector.tensor_mul(musq[:], mu[:], mu[:])
    nc.vector.tensor_scalar(var[:], ssq[:], 1.0 / DIM, None,
                            mybir.AluOpType.mult)
    nc.vector.tensor_sub(var[:], var[:], musq[:])
    nc.vector.tensor_scalar_add(var[:], var[:], float(EPS))
    sd = sb.tile([1, width], F32, name="sd")
    nc.scalar.activation(sd[:], var[:], AF.Sqrt)
    r = sb.tile([1, width], F32R, name="r")
    with nc.allow_low_precision(reason="f32r is fp32 storage"):
        nc.vector.reciprocal(r[:], sd[:])
    mu_b = sb.tile([128, width], F32R, name="mu_b", tag="mu_b")
    r_b = sb.tile([128, width], F32R, name="r_b", tag="r_b")
    nc.gpsimd.partition_broadcast(mu_b[:], mu[:])
    nc.gpsimd.partition_broadcast(r_b[:], r[:])
    xh = []
    for k in range(KD):
        xc = scratch.tile([128, width], F32, name="scr2", tag="xc")
        nc.vector.tensor_sub(xc[:], xt[:, k], mu_b[:])
        xhk = xh_pool.tile([128, width], out_dt, name=f"xh{k}", tag=f"xh{k}")
        nc.vector.tensor_mul(xhk[:], xc[:], r_b[:])
        xh.append(xhk)
    return xh


def _wslice(d, name, m, mm=128):
    """[DIM_in, n_out] weight dram -> lhsT tile view [128, KD_in, mm] for
    out-block m."""
    return d[name].rearrange("(k p) (mb mm) -> p k mb mm", p=128, mm=mm)[:, :, m]


NCHUNK = T // NQ  # 4


def _phase_a(nc, tc, d, Ktil, V5, Qtil, xt0, bias, ones16, ones32):
    """LN1 + K/V proj over all 4 chunks (redundant per core); Q proj on
    chunk 0 only. Writes the per-head attention layouts directly."""
    xT_t = d["xT"].rearrange("(k p) t -> p k t", p=128)
    with ExitStack() as actx:
        sb_ln = actx.enter_context(tc.tile_pool(name="sb_ln", bufs=2))
        scratch = actx.enter_context(tc.tile_pool(name="scratch", bufs=1))
        sb_xt = actx.enter_context(tc.tile_pool(name="sb_xt", bufs=2))
        sb_xh = actx.enter_context(tc.tile_pool(name="sb_xh", bufs=2))
        wstr = actx.enter_context(tc.tile_pool(name="wstr", bufs=2))
        ps_stat = actx.enter_context(tc.tile_pool(name="ps_stat", bufs=2, space="PSUM"))
        ps_mm = actx.enter_context(tc.tile_pool(name="ps_mm", bufs=4, space="PSUM"))

        wvT_v = d["wvT"].rearrange("(kh k p) (nb nn) -> p kh k nb nn",
                                   p=128, k=4, nn=NQ)
        for c in range(NCHUNK):
            if c == 0:
                xt = xt0
            else:
                xt = sb_xt.tile([128, KD, NQ], F16, name="xt")
                nc.sync.dma_start(out=xt[:],
                                  in_=xT_t[:, :, c * NQ:(c + 1) * NQ])
            xh = _ln_chunk(nc, sb_ln, scratch, ps_stat, sb_xh, xt,
                           ones16, ones32, NQ)

            # K projection (feature-major, split per head with 65-row layout)
            for m in range(KD):
                wk = wstr.tile([128, KD, 128], F16, name="wk", tag="wk")
                nc.sync.dma_start(out=wk[:], in_=_wslice(d, "wkT", m))
                kp = ps_mm.tile([128, NQ], F32, name="kp", tag="mm")
                for k in range(KD):
                    nc.tensor.matmul(kp[:], lhsT=wk[:, k], rhs=xh[k][:],
                                     start=(k == 0), stop=(k == KD - 1))
                nc.scalar.activation(
                    Ktil[2 * m][0:64, c * NQ:(c + 1) * NQ], kp[0:64, :],
                    AF.Identity, bias=bias["bk"][0:64, m:m + 1])
                nc.scalar.activation(
                    Ktil[2 * m + 1][0:64, c * NQ:(c + 1) * NQ], kp[64:128, :],
                    AF.Identity, bias=bias["bk"][64:128, m:m + 1])
            # V projection (token-major out, per-head 65-wide with ones col)
            V5a = V5.rearrange("p b (hh e) -> p b hh e", e=VW)
            for nb in range(2):
                wvs = []
                for kh in range(2):
                    wv = wstr.tile([128, 4, NQ], F16, name="wv", tag="wv")
                    nc.sync.dma_start(out=wv[:], in_=wvT_v[:, kh, :, nb])
                    wvs.append(wv)
                for t_sub in range(4):
                    blk = c * 4 + t_sub
                    vp = ps_mm.tile([128, NQ], F32, name="vp", tag="mm")
                    for k in range(KD):
                        nc.tensor.matmul(
                            vp[:], lhsT=xh[k][:, t_sub * 128:(t_sub + 1) * 128],
                            rhs=wvs[k // 4][:, k % 4], start=(k == 0), stop=(k == KD - 1))
                    vp3 = vp.rearrange("p (hh e) -> p hh e", e=DH)
                    nc.scalar.copy(
                        V5a[:, blk, nb * 8:(nb + 1) * 8, 0:DH], vp3[:])
            if c == 0:
                for m in range(KD):
                    wq = wstr.tile([128, KD, 128], F16, name="wq", tag="wk")
                    nc.sync.dma_start(out=wq[:], in_=_wslice(d, "wqT", m))
                    qp = ps_mm.tile([128, NQ], F32, name="qp", tag="mm")
                    for k in range(KD):
                        nc.tensor.matmul(qp[:], lhsT=wq[:, k], rhs=xh[k][:],
                                         start=(k == 0), stop=(k == KD - 1))
                    nc.scalar.activation(
                        Qtil[2 * m][0:64, :], qp[0:64, :], AF.Identity,
                        bias=bias["bq"][0:64, m:m + 1])
                    nc.scalar.activation(
                        Qtil[2 * m + 1][0:64, :], qp[64:128, :], AF.Identity,
                        bias=bias["bq"][64:128, m:m + 1])


def _phase_b(nc, tc, Ktil, V5, Qtil, ATTN_mbs):
    """Attention: two-pass max/exp, transposed A straight from EXP, ones-col
    denominator, 4-deep head pipeline."""
    with ExitStack() as bctx:
        psA = bctx.enter_context(tc.tile_pool(name="psA", bufs=1, space="PSUM"))
        psB = bctx.enter_context(tc.tile_pool(name="psB", bufs=2, space="PSUM"))
        psav = bctx.enter_context(tc.tile_pool(name="psav", bufs=2, space="PSUM"))
        sb_AT = bctx.enter_context(tc.tile_pool(name="sb_AT", bufs=2))
        sb_st = bctx.enter_context(tc.tile_pool(name="sb_st", bufs=2))
        sb_n = bctx.enter_context(tc.tile_pool(name="sb_n", bufs=2))

        AT = {}
        mxh = {}
        avt = {}

        def norm(h):
            av = avt.pop(h)
            rd = sb_n.tile([1, NQ], F32, name="rd", tag="rd")
            nc.vector.reciprocal(rd[:], av[64:65, :])
            rdb = sb_n.tile([64, NQ], F32, name="rdb", tag="rdb")
            nc.gpsimd.partition_broadcast(rdb[:], rd[:])
            mb, r0 = h // 2, (h % 2) * 64
            nc.vector.tensor_mul(ATTN_mbs[mb][r0:r0 + 64, :], av[0:64, :],
                                 rdb[:])

        for h in range(HEADS + 3):
            if h < HEADS:
                mxh[h] = sb_st.tile([128, 128], F16, name="mx", tag="mx")
            for j in range(NB):
                if h < HEADS and j % 4 == 0:
                    qt = j // 4
                    S = psA.tile([128, T], F32, name="S", tag="S")
                    for c in range(4):
                        nc.tensor.matmul(
                            S[:, c * NQ:(c + 1) * NQ],
                            lhsT=Qtil[h][0:65, qt * 128:(qt + 1) * 128],
                            rhs=Ktil[h][0:65, c * NQ:(c + 1) * NQ],
                            start=True, stop=True)
                    nc.vector.reduce_max(mxh[h][:, qt:qt + 1], S[:],
                                         axis=AX.X, negate=True)
                if 1 <= h < HEADS + 1:
                    hp = h - 1
                    if j == 0:
                        AT[hp] = sb_AT.tile([128, NB, NQ], F16, name="AT")
                    s2 = psB.tile([128, NQ], F32, name="s2", tag="s2")
                    nc.tensor.matmul(
                        s2[:], lhsT=Ktil[hp][0:65, j * 128:(j + 1) * 128],
                        rhs=Qtil[hp][0:65, :], start=True, stop=True)
                    nc.scalar.activation(AT[hp][:, j, :], s2[:], AF.Exp)
                if 2 <= h < HEADS + 2:
                    ha = h - 2
                    if j == 0:
                        avt[ha] = psav.tile([128, NQ], F32, name="av", tag="av")
                    nc.tensor.matmul(
                        avt[ha][0:VW, :],
                        lhsT=V5[:, j, ha * VW:(ha + 1) * VW],
                        rhs=AT[ha][:, j, :],
                        start=(j == 0), stop=(j == NB - 1))
                    if j == NB - 1:
                        AT.pop(ha)
            if h < HEADS:
                # negated maxes -> row 64 of Qtil[h] (XBAR transpose)
                mT = sb_st.tile([128, 128], F16, name="mT", tag="mT")
                nc.sync.dma_start(out=mT[:], in_=mxh.pop(h)[:], transpose=True)
                for qt in range(4):
                    nc.sync.dma_start(
                        out=Qtil[h][64:65, qt * 128:(qt + 1) * 128],
                        in_=mT[qt:qt + 1, 0:128])
            if h >= 3:
                norm(h - 3)


def _phase_c(nc, tc, d, ATTN_mbs, xt0, bias, ones32):
    """O proj + residual + LN2 + FF + output store."""
    with ExitStack() as cctx:
        sb_ln2 = cctx.enter_context(tc.tile_pool(name="sb_ln2", bufs=2))
        scr2 = cctx.enter_context(tc.tile_pool(name="scr2", bufs=2))
        sb_u = cctx.enter_context(tc.tile_pool(name="sb_u", bufs=1))
        wstr2 = cctx.enter_context(tc.tile_pool(name="wstr2", bufs=4))
        ps_stat2 = cctx.enter_context(tc.tile_pool(name="ps_stat2", bufs=1, space="PSUM"))
        ps_mm2 = cctx.enter_context(tc.tile_pool(name="ps_mm2", bufs=6, space="PSUM"))

        u_sb = sb_u.tile([128, KD, NQ], F32R, name="u_sb")
        for m in range(KD):
            wot = wstr2.tile([128, KD, 128], F16, name="wo", tag="wsm")
            nc.sync.dma_start(out=wot[:], in_=_wslice(d, "woT", m))
            op = ps_mm2.tile([128, NQ], F32, name="op", tag="mm")
            for k in range(KD):
                nc.tensor.matmul(op[:], lhsT=wot[:, k], rhs=ATTN_mbs[k][:],
                                 start=(k == 0), stop=(k == KD - 1))
            upre = scr2.tile([128, NQ], F32, name="upre", tag="scr")
            nc.vector.tensor_add(upre[:], op[:], xt0[:, m])
            nc.scalar.activation(u_sb[:, m], upre[:], AF.Identity,
                                 bias=bias["bo"][:, m:m + 1])
        uh = _ln_chunk(nc, sb_ln2, scr2, ps_stat2, sb_u, u_sb, ones32,
                       ones32, NQ, out_dt=F16)
        H_sb = sb_u.tile([128, FF // 128, NQ], F16, name="H_sb")
        for m in range(FF // 128):
            w1t = wstr2.tile([128, KD, 128], F16, name="w1", tag="wsm")
            nc.sync.dma_start(out=w1t[:], in_=_wslice(d, "w1T", m))
            h1 = ps_mm2.tile([128, NQ], F32, name="h1", tag="mm")
            for k in range(KD):
                nc.tensor.matmul(h1[:], lhsT=w1t[:, k], rhs=uh[k][:],
                                 start=(k == 0), stop=(k == KD - 1))
            nc.scalar.activation(H_sb[:, m], h1[:], AF.Gelu,
                                 bias=bias["b1"][:, m:m + 1])
        w2T_v = d["w2T"].rearrange("(kh k p) (mb mm) -> p kh k mb mm",
                                   p=128, k=8, mm=128)
        for m in range(KD):
            f2 = ps_mm2.tile([128, NQ], F32, name="f2", tag="mm")
            for kh in range(4):
                w2 = wstr2.tile([128, 8, 128], F16, name="w2", tag="w2")
                nc.sync.dma_start(out=w2[:], in_=w2T_v[:, kh, :, m])
                for k in range(8):
                    nc.tensor.matmul(f2[:], lhsT=w2[:, k], rhs=H_sb[:, kh * 8 + k],
                                     start=(kh == 0 and k == 0),
                                     stop=(kh == 3 and k == 7))
            opre = scr2.tile([128, NQ], F32, name="opre", tag="scr")
            nc.vector.tensor_add(opre[:], f2[:], u_sb[:, m])
            oout = scr2.tile([128, NQ], F32, name="oout", tag="scr")
            nc.scalar.activation(oout[:], opre[:], AF.Identity,
                                 bias=bias["b2"][:, m:m + 1])
            nc.sync.dma_start(out=d["yT"][m * 128:(m + 1) * 128, :], in_=oout[:])


def _body(nc, tc, d):
    ctx = ExitStack()
    with ctx:
        const = ctx.enter_context(tc.tile_pool(name="const", bufs=1))
        ones_blk = const.tile([128, 128], F32, name="ones_blk")
        nc.vector.memset(ones_blk[:], 1.0)
        ones32 = const.tile([128, 1], F32R, name="ones32")
        nc.vector.tensor_copy(ones32[:], ones_blk[:, 0:1])
        ones16 = const.tile([128, 1], F16, name="ones16")
        nc.vector.tensor_copy(ones16[:], ones_blk[:, 0:1])

        xt0 = const.tile([128, KD, NQ], F16, name="xt0")
        nc.sync.dma_start(
            out=xt0[:],
            in_=d["xT"].rearrange("(k p) t -> p k t", p=128)[:, :, 0:NQ])

        bias = {}
        for nm, n in [("bq", DIM), ("bk", DIM), ("bo", DIM), ("b1", FF), ("b2", DIM)]:
            t = const.tile([128, n // 128], F32, name=f"sb_{nm}")
            nc.sync.dma_start(out=t[:], in_=d[nm].rearrange("(m p) -> p m", p=128))
            bias[nm] = t

        # long-lived activations
        ATTN_mbs = [const.tile([128, NQ], F16, name=f"ATTN_{i}") for i in range(KD)]

        with tc.tile_pool(name="attn_mem", bufs=1) as am:
            Ktil = [am.tile([128, T], F16, name=f"Kt_{h}") for h in range(HEADS)]
            Qtil = [am.tile([128, NQ], F16, name=f"Qt_{h}") for h in range(HEADS)]
            V5 = am.tile([128, NB, HEADS * VW], F16, name="V5")
            # ones row (K), zero row (Q), ones col (V) via one memset + DMAs
            with tc.tile_pool(name="init", bufs=1) as ip:
                orow = ip.tile([1, T], F16, name="orow")
                zrow = ip.tile([1, NQ], F16, name="zrow")
                nc.vector.memset(orow[:], 1.0)
                nc.vector.memset(zrow[:], 0.0)
                for h in range(HEADS):
                    nc.sync.dma_start(out=Ktil[h][64:65, :], in_=orow[:])
                    nc.sync.dma_start(out=Qtil[h][64:65, :], in_=zrow[:])
            V5o = V5.rearrange("p b (hh e) -> p b hh e", e=VW)
            ones_c = bass.AP(tensor=ones_blk.tensor, offset=ones_blk.offset,
                             ap=[list(ones_blk.ap[0])] + [[0, NB], [0, HEADS], [0, 1]])
            nc.vector.tensor_copy(V5o[:, :, :, DH:DH + 1], ones_c)

            _phase_a(nc, tc, d, Ktil, V5, Qtil, xt0, bias, ones16, ones32)
            _phase_b(nc, tc, Ktil, V5, Qtil, ATTN_mbs)
        _phase_c(nc, tc, d, ATTN_mbs, xt0, bias, ones32)


def _build():
    nc = bacc.Bacc("TRN2", target_bir_lowering=False, debug=False,
                   num_devices=N_CORES)
    d = {}
    d["xT"] = nc.dram_tensor("xT", [DIM, T], F16, kind="ExternalInput").ap()
    d["wqT"] = nc.dram_tensor("wqT", [DIM, DIM], F16, kind="ExternalInput").ap()
    d["wkT"] = nc.dram_tensor("wkT", [DIM, DIM], F16, kind="ExternalInput").ap()
    d["wvT"] = nc.dram_tensor("wvT", [DIM, DIM], F16, kind="ExternalInput").ap()
    d["woT"] = nc.dram_tensor("woT", [DIM, DIM], F16, kind="ExternalInput").ap()
    d["w1T"] = nc.dram_tensor("w1T", [DIM, FF], F16, kind="ExternalInput").ap()
    d["w2T"] = nc.dram_tensor("w2T", [FF, DIM], F16, kind="ExternalInput").ap()
    for nm, n in [("bq", DIM), ("bk", DIM), ("bo", DIM), ("b1", FF), ("b2", DIM)]:
        d[nm] = nc.dram_tensor(nm, [n], F32, kind="ExternalInput").ap()
    d["yT"] = nc.dram_tensor("yT", [DIM, NQ], F32, kind="ExternalOutput").ap()
    with tile.TileContext(nc) as tc:
        _body(nc, tc, d)
    nc.compile()
    return nc


def _in_maps(inputs):
    x = inputs["x"].astype(np.float32)
    B = x.shape[0]
    w = _prep_weights(inputs)
    per_batch = N_CORES // B
    maps = []
    for c in range(N_CORES):
        b, chunk = divmod(c, per_batch)
        xT = np.ascontiguousarray(
            np.roll(x[b].T, -chunk * NQ, axis=1)).astype(np.float16)
        m = {"xT": xT}
        m.update(w)
        maps.append(m)
    return maps


def kernel(**inputs) -> np.ndarray:
    inputs = {k: np.asarray(v) for k, v in inputs.items()}
    x = inputs["x"].astype(np.float32)
    B, N, D = x.shape  # (2, 2048, 1024)

    if "nc" not in _cache:
        _cache["nc"] = _build()
    nc = _cache["nc"]

    res = run_bass_kernel_spmd(nc, _in_maps(inputs), core_ids=list(range(N_CORES)))
    per_batch = N_CORES // B
    out = np.empty((B, N, D), dtype=np.float32)
    for c in range(N_CORES):
        b, chunk = divmod(c, per_batch)
        out[b, chunk * NQ:(chunk + 1) * NQ, :] = res.results[c]["yT"].T
    return out
